# revision 17
# baseline (speedup 1.0000x reference)
"""Trainium2 Bass kernel for nn_Attention_48541720379807.

Multi-head attention (N=8 heads, H=128) with per-head K/Q projections,
softmax over projected keys, attention applied to projected keys, head
concat, and an output Linear.  B=8, L=2048, E=1024.

Sharding: pure data parallel - batch element b -> NeuronCore b.  Each core
computes its full batch slice including the output projection; the host
slices inputs and stacks outputs.  No collectives.

v3 (all-bf16 operands, fp32 PSUM accumulation, no DRAM scratch,
per-head phase interleaving so the PE never waits on the exp engine):
  per head n:  A_k[n], A_q[n] (projections from SBUF-resident kT/qT),
               16 PE transposes -> kx_nat, then 4 q-blocks of
               score (PE) -> exp (ACT, per-chunk 128x512, bf16 out)
               -> AV accumulate + ones-denominator matmuls packed 4-wide
               into PE column groups -> partition_all_reduce ->
               reciprocal_approx_fast -> broadcast -> normalize into the
               previous head's qxT tile (dead by then).
  final C:     y(L,E) = sum_c on_c[qt].T @ pwT_c + b, all from SBUF.

The A-phase matmuls of head n+1 interleave (via the Tile scheduler) with
head n's attention, keeping the PE busy while ACT chews through exp.
"""

import math

import numpy as np

B, L, E, N, H = 8, 2048, 1024, 8, 128
NCORES = 8
QBLK = 512          # q block width in phase B
KCH = L // 128      # 16 k chunks / k tiles
ECH = E // 128      # 8 e chunks
SCALE = 1.0 / math.sqrt(H)

MODE = "bf16"

_CACHE = {}
_last_in_maps = None


def _build(mode):
    from concourse import bacc, bass_isa
    import concourse.mybir as mybir
    from concourse.tile import TileContext
    from concourse.masks import make_identity

    f32 = mybir.dt.float32
    bf16 = mybir.dt.bfloat16

    nc = bacc.Bacc("TRN2", target_bir_lowering=False, debug=False,
                   num_devices=NCORES)

    kT_d = nc.dram_tensor("kT", [E, L], bf16, kind="ExternalInput")
    qT_d = nc.dram_tensor("qT", [E, L], bf16, kind="ExternalInput")
    wk_d = nc.dram_tensor("wk", [E, N * H], bf16, kind="ExternalInput")
    wq_d = nc.dram_tensor("wq", [E, N * H], bf16, kind="ExternalInput")
    pwT_d = nc.dram_tensor("pwT", [N * H, E], bf16, kind="ExternalInput")
    pb_d = nc.dram_tensor("pb", [1, E], bf16, kind="ExternalInput")
    y_d = nc.dram_tensor("y", [L, E], f32, kind="ExternalOutput")

    with TileContext(nc) as tc:
        with (
            tc.tile_pool(name="const", bufs=1) as const,
            tc.tile_pool(name="kq", bufs=1) as kq,       # kT/qT tiles
            tc.tile_pool(name="wp", bufs=1) as wp,       # wk/wq, reused for pwT
            tc.tile_pool(name="kxp", bufs=1) as kxp,     # kxT_all (8x 128x2048)
            tc.tile_pool(name="qxp", bufs=1) as qxp,     # qxT_all, reused for on
            tc.tile_pool(name="onp", bufs=1) as onp,     # head-0 on tile
            tc.tile_pool(name="kxn", bufs=1) as kxn,     # kx_nat x2 (dedicated)
            tc.tile_pool(name="expp", bufs=7) as expp,   # exp chunks (128,512) bf16
            tc.tile_pool(name="dnp", bufs=1) as dnp,     # den4 / dbc / dbc2 f32
            tc.tile_pool(name="small", bufs=1) as small,
            tc.tile_pool(name="ysb", bufs=2) as ysb,
            tc.tile_pool(name="psA", bufs=2, space="PSUM") as psA,   # scores, C
            tc.tile_pool(name="psB", bufs=2, space="PSUM") as psB,   # phase A
            tc.tile_pool(name="psT", bufs=1, space="PSUM") as psT,   # transposes
            tc.tile_pool(name="psO", bufs=2, space="PSUM") as psO,   # AV accum
            tc.tile_pool(name="psD", bufs=1, space="PSUM") as psD,   # denominator
        ):
            ident = const.tile([128, 128], bf16)
            make_identity(nc, ident)
            ones_f = const.tile([128, 1], f32)
            nc.any.memset(ones_f[:], 1.0)
            ones = const.tile([128, 1], bf16)
            nc.vector.tensor_copy(ones[:], ones_f[:])
            pb_sb = const.tile([1, E], bf16)
            nc.sync.dma_start(out=pb_sb[:], in_=pb_d[:])
            pb_bc = const.tile([128, E], bf16)
            nc.gpsimd.partition_broadcast(pb_bc[:], pb_sb[:])

            # only partitions {0,32,64,96} are ever written by the den
            # matmuls; zero the rest once so partition_all_reduce over 128
            # channels sees clean rows.
            den4 = dnp.tile([128, QBLK], f32, tag="den4")
            nc.any.memset(den4[:], 0.0)

            # HAM warm-up: keep the PE busy with tiny matmuls while the
            # first kT/wk DMAs are in flight, so phase A starts at the full
            # 2.4 GHz clock instead of the cold 1.2 GHz throttle.
            warm = psD.tile([128, QBLK], f32, tag="psD")
            for i in range(70):
                nc.tensor.matmul(warm[0:1, 0:128], ones[:], ident[:],
                                 start=(i == 0), stop=(i == 69))

            # ---------------- input loads (k-side first) ----------------
            wk_tiles, kT_tiles, wq_tiles, qT_tiles = [], [], [], []
            for ec in range(ECH):
                wt = wp.tile([128, N * H], bf16, tag=f"wk{ec}")
                nc.sync.dma_start(out=wt[:], in_=wk_d[ec * 128:(ec + 1) * 128, :])
                wk_tiles.append(wt)
                kt = kq.tile([128, L], bf16, tag=f"kt{ec}")
                nc.sync.dma_start(out=kt[:], in_=kT_d[ec * 128:(ec + 1) * 128, :])
                kT_tiles.append(kt)
            for ec in range(ECH):
                wt = wp.tile([128, N * H], bf16, tag=f"wq{ec}")
                nc.sync.dma_start(out=wt[:], in_=wq_d[ec * 128:(ec + 1) * 128, :])
                wq_tiles.append(wt)
                qt = kq.tile([128, L], bf16, tag=f"qt{ec}")
                nc.sync.dma_start(out=qt[:], in_=qT_d[ec * 128:(ec + 1) * 128, :])
                qT_tiles.append(qt)

            def phase_a(n, src_tiles, w_tiles, dst):
                hs = slice(n * H, (n + 1) * H)
                for lb in range(4):
                    ls = slice(lb * 512, (lb + 1) * 512)
                    ps = psB.tile([128, 512], f32, tag="psB")
                    for ec in range(ECH):
                        nc.tensor.matmul(
                            ps[:], w_tiles[ec][:, hs], src_tiles[ec][:, ls],
                            start=(ec == 0), stop=(ec == ECH - 1))
                    nc.vector.tensor_copy(dst[:, ls], ps[:])

            def make_kx_nat(n, kxT):
                kx_nat = kxn.tile([128, KCH * H], bf16, tag=f"kxn{n % 2}")
                for grp in range(KCH // 4):
                    pt = psT.tile([128, 512], bf16, tag="pt")
                    for j in range(4):
                        kc = grp * 4 + j
                        nc.tensor.transpose(
                            pt[:, j * 128:(j + 1) * 128],
                            kxT[:, kc * 128:(kc + 1) * 128], ident[:])
                    nc.vector.tensor_copy(
                        kx_nat[:, grp * 512:(grp + 1) * 512], pt[:])
                return kx_nat

            def do_qblk(kxT, qxT, kx_nat, on, qb):
                qs = slice(qb * QBLK, (qb + 1) * QBLK)
                ets = []
                for kc in range(KCH):
                    ps_s = psA.tile([128, QBLK], f32, tag="psA")
                    nc.tensor.matmul(
                        ps_s[:], kxT[:, kc * 128:(kc + 1) * 128],
                        qxT[:, qs], start=True, stop=True)
                    et = expp.tile([128, QBLK], bf16, tag="expt")
                    nc.scalar.activation(
                        et[:], ps_s[:],
                        mybir.ActivationFunctionType.Exp, scale=SCALE)
                    ets.append(et)
                ps_o = psO.tile([128, QBLK], f32, tag="psO")
                ps_d = psD.tile([128, QBLK], f32, tag="psD")
                for g in range(4):
                    for j in range(4):
                        kc = 4 * g + j
                        nc.tensor.matmul(
                            ps_o[:], kx_nat[:, kc * H:(kc + 1) * H],
                            ets[kc][:],
                            start=(kc == 0), stop=(kc == KCH - 1))
                    for j in range(4):
                        kc = 4 * g + j
                        nc.tensor.matmul(
                            ps_d[32 * j:32 * j + 1, :], ones[:],
                            ets[kc][:],
                            start=(g == 0), stop=(g == 3),
                            tile_position=(0, 32 * j))
                for j in range(4):
                    nc.vector.tensor_copy(
                        den4[32 * j:32 * j + 1, :],
                        ps_d[32 * j:32 * j + 1, :])
                dbc = dnp.tile([128, QBLK], f32, tag="dbc")
                nc.gpsimd.partition_all_reduce(
                    dbc[:], den4[:], channels=128,
                    reduce_op=bass_isa.ReduceOp.add)
                d_rc = small.tile([1, QBLK], f32, tag="drc")
                nc.vector.reciprocal_approx_fast(d_rc[:], dbc[0:1, :])
                d_bc = dnp.tile([128, QBLK], f32, tag="dbc2")
                nc.gpsimd.partition_broadcast(d_bc[:], d_rc[:])
                nc.vector.tensor_mul(on[:, qs], ps_o[:], d_bc[:])

            on_tiles = []
            kxT_all = []
            for n in range(N):
                # ---- phase A for this head ----
                with nc.named_scope(f"A{n}"):
                    kxT = kxp.tile([128, L], bf16, tag=f"kx{n}")
                    phase_a(n, kT_tiles, wk_tiles, kxT)
                    kxT_all.append(kxT)
                    qxT = qxp.tile([128, L], bf16, tag=f"qx{n}")
                    phase_a(n, qT_tiles, wq_tiles, qxT)

                # ---- attention for this head ----
                with nc.named_scope(f"B{n}"):
                    kx_nat = make_kx_nat(n, kxT)
                    if n == 0:
                        on = onp.tile([128, L], bf16, tag="on0")
                    else:
                        on = qxp.tile([128, L], bf16, tag=f"qx{n - 1}")
                    for qb in range(L // QBLK):
                        do_qblk(kxT, qxT, kx_nat, on, qb)
                    on_tiles.append(on)

            # pwT into the wk tile slots (dead after A_k[7])
            pw_tiles = []
            for c in range(N):
                pwt = wp.tile([128, E], bf16, tag=f"wk{c}")
                nc.sync.dma_start(out=pwt[:], in_=pwT_d[c * 128:(c + 1) * 128, :])
                pw_tiles.append(pwt)

            # ---------------- Phase C ----------------
            with nc.named_scope("C"):
                for qt in range(L // 128):
                    y_sb = ysb.tile([128, E], f32, tag="ysb")
                    for eb in range(2):
                        ps_y = psA.tile([128, 512], f32, tag="psA")
                        for c in range(N):
                            nc.tensor.matmul(
                                ps_y[:],
                                on_tiles[c][:, qt * 128:(qt + 1) * 128],
                                pw_tiles[c][:, eb * 512:(eb + 1) * 512],
                                start=(c == 0), stop=(c == N - 1))
                        nc.vector.tensor_add(
                            y_sb[:, eb * 512:(eb + 1) * 512], ps_y[:],
                            pb_bc[:, eb * 512:(eb + 1) * 512])
                    nc.sync.dma_start(out=y_d[qt * 128:(qt + 1) * 128, :],
                                      in_=y_sb[:])

    nc.compile()
    return nc


def _get_program(mode=MODE):
    if mode not in _CACHE:
        _CACHE[mode] = _build(mode)
    return _CACHE[mode]


def kernel(k, q, w_kx, w_qx, proj_w, proj_b, mode=MODE):
    import ml_dtypes
    from concourse.bass_utils import run_bass_kernel_spmd

    bf16 = ml_dtypes.bfloat16
    k = np.asarray(k, dtype=np.float32)
    q = np.asarray(q, dtype=np.float32)
    w_kx = np.asarray(w_kx, dtype=np.float32)
    w_qx = np.asarray(w_qx, dtype=np.float32)
    proj_w = np.asarray(proj_w, dtype=np.float32)
    proj_b = np.asarray(proj_b, dtype=np.float32)

    wk = np.ascontiguousarray(
        w_kx.transpose(1, 0, 2).reshape(E, N * H)).astype(bf16)   # (e, n*h)
    wq = np.ascontiguousarray(
        w_qx.transpose(1, 0, 2).reshape(E, N * H)).astype(bf16)
    pwT = np.ascontiguousarray(proj_w.T).astype(bf16)
    pb = np.ascontiguousarray(proj_b.reshape(1, E)).astype(bf16)

    in_maps = []
    for b in range(NCORES):
        in_maps.append({
            "kT": np.ascontiguousarray(k[b].T).astype(bf16),
            "qT": np.ascontiguousarray(q[b].T).astype(bf16),
            "wk": wk,
            "wq": wq,
            "pwT": pwT,
            "pb": pb,
        })

    global _last_in_maps
    _last_in_maps = in_maps
    nc = _get_program(mode)
    res = run_bass_kernel_spmd(nc, in_maps, list(range(NCORES)))
    out = np.stack([res.results[b]["y"] for b in range(NCORES)], axis=0)
    return out.astype(np.float32)


# revision 18
# speedup vs baseline: 1.1760x; 1.1760x over previous
"""Trainium2 Bass kernel for nn_Attention_48541720379807.

Multi-head attention (N=8 heads, H=128) with per-head K/Q projections,
softmax over projected keys, attention applied to projected keys, head
concat, and an output Linear.  B=8, L=2048, E=1024.

Sharding: pure data parallel - batch element b -> NeuronCore b.  Each core
computes its full batch slice including the output projection; the host
slices inputs and stacks outputs.  No collectives.

v3 (all-bf16 operands, fp32 PSUM accumulation, no DRAM scratch,
per-head phase interleaving so the PE never waits on the exp engine):
  per head n:  A_k[n], A_q[n] (projections from SBUF-resident kT/qT),
               16 PE transposes -> kx_nat, then 4 q-blocks of
               score (PE) -> exp (ACT, per-chunk 128x512, bf16 out)
               -> AV accumulate + ones-denominator matmuls packed 4-wide
               into PE column groups -> partition_all_reduce ->
               reciprocal_approx_fast -> broadcast -> normalize into the
               previous head's qxT tile (dead by then).
  final C:     y(L,E) = sum_c on_c[qt].T @ pwT_c + b, all from SBUF.

The A-phase matmuls of head n+1 interleave (via the Tile scheduler) with
head n's attention, keeping the PE busy while ACT chews through exp.
"""

import math

import numpy as np

B, L, E, N, H = 8, 2048, 1024, 8, 128
NCORES = 8
QBLK = 512          # q block width in phase B
KCH = L // 128      # 16 k chunks / k tiles
ECH = E // 128      # 8 e chunks
SCALE = 1.0 / math.sqrt(H)

MODE = "bf16"

_CACHE = {}
_last_in_maps = None


def _build(mode):
    from concourse import bacc, bass_isa
    import concourse.mybir as mybir
    from concourse.tile import TileContext
    from concourse.masks import make_identity

    f32 = mybir.dt.float32
    bf16 = mybir.dt.bfloat16

    nc = bacc.Bacc("TRN2", target_bir_lowering=False, debug=False,
                   num_devices=NCORES)

    kT_d = nc.dram_tensor("kT", [E, L], bf16, kind="ExternalInput")
    qT_d = nc.dram_tensor("qT", [E, L], bf16, kind="ExternalInput")
    wk_d = nc.dram_tensor("wk", [E, N * H], bf16, kind="ExternalInput")
    wq_d = nc.dram_tensor("wq", [E, N * H], bf16, kind="ExternalInput")
    pwT_d = nc.dram_tensor("pwT", [N * H, E], bf16, kind="ExternalInput")
    pb_d = nc.dram_tensor("pb", [1, E], bf16, kind="ExternalInput")
    y_d = nc.dram_tensor("y", [L, E], f32, kind="ExternalOutput")

    with TileContext(nc) as tc:
        with (
            tc.tile_pool(name="const", bufs=1) as const,
            tc.tile_pool(name="kq", bufs=1) as kq,       # kT/qT tiles
            tc.tile_pool(name="wp", bufs=1) as wp,       # wk/wq, reused for pwT
            tc.tile_pool(name="kxp", bufs=1) as kxp,     # kxT_all (8x 128x2048)
            tc.tile_pool(name="qxp", bufs=1) as qxp,     # qxT_all, reused for on
            tc.tile_pool(name="onp", bufs=1) as onp,     # head-0 on tile
            tc.tile_pool(name="kxn", bufs=1) as kxn,     # kx_nat x2 (dedicated)
            tc.tile_pool(name="expp", bufs=7) as expp,   # exp chunks (128,512) bf16
            tc.tile_pool(name="dnp", bufs=1) as dnp,     # den4 / dbc / dbc2 f32
            tc.tile_pool(name="small", bufs=1) as small,
            tc.tile_pool(name="ysb", bufs=2) as ysb,
            tc.tile_pool(name="psA", bufs=2, space="PSUM") as psA,   # scores, C
            tc.tile_pool(name="psB", bufs=2, space="PSUM") as psB,   # phase A
            tc.tile_pool(name="psT", bufs=1, space="PSUM") as psT,   # transposes
            tc.tile_pool(name="psO", bufs=2, space="PSUM") as psO,   # AV accum
            tc.tile_pool(name="psD", bufs=1, space="PSUM") as psD,   # denominator
        ):
            ident = const.tile([128, 128], bf16)
            make_identity(nc, ident)
            ones_f = const.tile([128, 1], f32)
            nc.any.memset(ones_f[:], 1.0)
            ones = const.tile([128, 1], bf16)
            nc.vector.tensor_copy(ones[:], ones_f[:])
            pb_sb = const.tile([1, E], bf16)
            nc.sync.dma_start(out=pb_sb[:], in_=pb_d[:])
            pb_bc = const.tile([128, E], bf16)
            nc.gpsimd.partition_broadcast(pb_bc[:], pb_sb[:])

            # only partitions {0,32,64,96} are ever written by the den
            # matmuls; zero the rest once so partition_all_reduce over 128
            # channels sees clean rows.
            den4 = dnp.tile([128, QBLK], f32, tag="den4")
            nc.any.memset(den4[:], 0.0)

            # ---------------- input loads (k-side first) ----------------
            wk_tiles, kT_tiles, wq_tiles, qT_tiles = [], [], [], []
            for ec in range(ECH):
                wt = wp.tile([128, N * H], bf16, tag=f"wk{ec}")
                nc.sync.dma_start(out=wt[:], in_=wk_d[ec * 128:(ec + 1) * 128, :])
                wk_tiles.append(wt)
                kt = kq.tile([128, L], bf16, tag=f"kt{ec}")
                nc.sync.dma_start(out=kt[:], in_=kT_d[ec * 128:(ec + 1) * 128, :])
                kT_tiles.append(kt)
            for ec in range(ECH):
                wt = wp.tile([128, N * H], bf16, tag=f"wq{ec}")
                nc.sync.dma_start(out=wt[:], in_=wq_d[ec * 128:(ec + 1) * 128, :])
                wq_tiles.append(wt)
                qt = kq.tile([128, L], bf16, tag=f"qt{ec}")
                nc.sync.dma_start(out=qt[:], in_=qT_d[ec * 128:(ec + 1) * 128, :])
                qT_tiles.append(qt)

            def phase_a(n, src_tiles, w_tiles, dst):
                hs = slice(n * H, (n + 1) * H)
                for lb in range(4):
                    ls = slice(lb * 512, (lb + 1) * 512)
                    ps = psB.tile([128, 512], f32, tag="psB")
                    for ec in range(ECH):
                        nc.tensor.matmul(
                            ps[:], w_tiles[ec][:, hs], src_tiles[ec][:, ls],
                            start=(ec == 0), stop=(ec == ECH - 1))
                    nc.vector.tensor_copy(dst[:, ls], ps[:])

            def make_kx_nat(n, kxT):
                kx_nat = kxn.tile([128, KCH * H], bf16, tag=f"kxn{n % 2}")
                for grp in range(KCH // 4):
                    pt = psT.tile([128, 512], bf16, tag="pt")
                    for j in range(4):
                        kc = grp * 4 + j
                        nc.tensor.transpose(
                            pt[:, j * 128:(j + 1) * 128],
                            kxT[:, kc * 128:(kc + 1) * 128], ident[:])
                    nc.vector.tensor_copy(
                        kx_nat[:, grp * 512:(grp + 1) * 512], pt[:])
                return kx_nat

            def do_qblk(kxT, qxT, kx_nat, on, qb):
                qs = slice(qb * QBLK, (qb + 1) * QBLK)
                ets = []
                for kc in range(KCH):
                    ps_s = psA.tile([128, QBLK], f32, tag="psA")
                    nc.tensor.matmul(
                        ps_s[:], kxT[:, kc * 128:(kc + 1) * 128],
                        qxT[:, qs], start=True, stop=True)
                    et = expp.tile([128, QBLK], bf16, tag="expt")
                    nc.scalar.activation(
                        et[:], ps_s[:],
                        mybir.ActivationFunctionType.Exp, scale=SCALE)
                    ets.append(et)
                ps_o = psO.tile([128, QBLK], f32, tag="psO")
                ps_d = psD.tile([128, QBLK], f32, tag="psD")
                for g in range(4):
                    for j in range(4):
                        kc = 4 * g + j
                        nc.tensor.matmul(
                            ps_o[:], kx_nat[:, kc * H:(kc + 1) * H],
                            ets[kc][:],
                            start=(kc == 0), stop=(kc == KCH - 1))
                    for j in range(4):
                        kc = 4 * g + j
                        nc.tensor.matmul(
                            ps_d[32 * j:32 * j + 1, :], ones[:],
                            ets[kc][:],
                            start=(g == 0), stop=(g == 3),
                            tile_position=(0, 32 * j))
                for j in range(4):
                    nc.vector.tensor_copy(
                        den4[32 * j:32 * j + 1, :],
                        ps_d[32 * j:32 * j + 1, :])
                dbc = dnp.tile([128, QBLK], f32, tag="dbc")
                nc.gpsimd.partition_all_reduce(
                    dbc[:], den4[:], channels=128,
                    reduce_op=bass_isa.ReduceOp.add)
                d_rc = small.tile([1, QBLK], f32, tag="drc")
                nc.vector.reciprocal_approx_fast(d_rc[:], dbc[0:1, :])
                d_bc = dnp.tile([128, QBLK], f32, tag="dbc2")
                nc.gpsimd.partition_broadcast(d_bc[:], d_rc[:])
                nc.vector.tensor_mul(on[:, qs], ps_o[:], d_bc[:])

            on_tiles = []
            kxT_all = []
            for n in range(N):
                # ---- phase A for this head ----
                with nc.named_scope(f"A{n}"):
                    kxT = kxp.tile([128, L], bf16, tag=f"kx{n}")
                    phase_a(n, kT_tiles, wk_tiles, kxT)
                    kxT_all.append(kxT)
                    qxT = qxp.tile([128, L], bf16, tag=f"qx{n}")
                    phase_a(n, qT_tiles, wq_tiles, qxT)

                # ---- attention for this head ----
                with nc.named_scope(f"B{n}"):
                    kx_nat = make_kx_nat(n, kxT)
                    if n == 0:
                        on = onp.tile([128, L], bf16, tag="on0")
                    else:
                        on = qxp.tile([128, L], bf16, tag=f"qx{n - 1}")
                    for qb in range(L // QBLK):
                        do_qblk(kxT, qxT, kx_nat, on, qb)
                    on_tiles.append(on)

            # pwT into the wk tile slots (dead after A_k[7])
            pw_tiles = []
            for c in range(N):
                pwt = wp.tile([128, E], bf16, tag=f"wk{c}")
                nc.sync.dma_start(out=pwt[:], in_=pwT_d[c * 128:(c + 1) * 128, :])
                pw_tiles.append(pwt)

            # ---------------- Phase C ----------------
            with nc.named_scope("C"):
                for qt in range(L // 128):
                    y_sb = ysb.tile([128, E], f32, tag="ysb")
                    for eb in range(2):
                        ps_y = psA.tile([128, 512], f32, tag="psA")
                        for c in range(N):
                            nc.tensor.matmul(
                                ps_y[:],
                                on_tiles[c][:, qt * 128:(qt + 1) * 128],
                                pw_tiles[c][:, eb * 512:(eb + 1) * 512],
                                start=(c == 0), stop=(c == N - 1))
                        nc.vector.tensor_add(
                            y_sb[:, eb * 512:(eb + 1) * 512], ps_y[:],
                            pb_bc[:, eb * 512:(eb + 1) * 512])
                    nc.sync.dma_start(out=y_d[qt * 128:(qt + 1) * 128, :],
                                      in_=y_sb[:])

    nc.compile()
    return nc


def _get_program(mode=MODE):
    if mode not in _CACHE:
        _CACHE[mode] = _build(mode)
    return _CACHE[mode]


def kernel(k, q, w_kx, w_qx, proj_w, proj_b, mode=MODE):
    import ml_dtypes
    from concourse.bass_utils import run_bass_kernel_spmd

    bf16 = ml_dtypes.bfloat16
    k = np.asarray(k, dtype=np.float32)
    q = np.asarray(q, dtype=np.float32)
    w_kx = np.asarray(w_kx, dtype=np.float32)
    w_qx = np.asarray(w_qx, dtype=np.float32)
    proj_w = np.asarray(proj_w, dtype=np.float32)
    proj_b = np.asarray(proj_b, dtype=np.float32)

    wk = np.ascontiguousarray(
        w_kx.transpose(1, 0, 2).reshape(E, N * H)).astype(bf16)   # (e, n*h)
    wq = np.ascontiguousarray(
        w_qx.transpose(1, 0, 2).reshape(E, N * H)).astype(bf16)
    pwT = np.ascontiguousarray(proj_w.T).astype(bf16)
    pb = np.ascontiguousarray(proj_b.reshape(1, E)).astype(bf16)

    in_maps = []
    for b in range(NCORES):
        in_maps.append({
            "kT": np.ascontiguousarray(k[b].T).astype(bf16),
            "qT": np.ascontiguousarray(q[b].T).astype(bf16),
            "wk": wk,
            "wq": wq,
            "pwT": pwT,
            "pb": pb,
        })

    global _last_in_maps
    _last_in_maps = in_maps
    nc = _get_program(mode)
    res = run_bass_kernel_spmd(nc, in_maps, list(range(NCORES)))
    out = np.stack([res.results[b]["y"] for b in range(NCORES)], axis=0)
    return out.astype(np.float32)
